# revision 19
# baseline (speedup 1.0000x reference)
"""Trainium2 Bass kernel for a 3-layer GCN (PyG GCNConv semantics) + trace(A@A) reg scalar.

Strategy (8 NeuronCores, one SPMD program):
  - Nodes are ranked by degree (desc) and dealt round-robin to the 8 cores so
    every core's window-by-window workload is statistically identical; this
    lets one static SPMD program (uniform chunk counts per window) serve all
    cores with per-core index *data*.
  - H (layer input, node-major [40000,128] f32) is replicated in each core's
    DRAM (input x is uploaded pre-permuted; later layers use AllGather).
  - Per edge (dst-partitioned, dst-sorted into 32-node windows) the 512B
    feature row of src is fetched with gpsimd.dma_gather (1 descriptor/edge,
    int16 indices; two base offsets 0 / N-32768 cover all 40000 rows).
  - segment-sum = PE matmuls: psum[feat,32] += msg_chunk[128e,128f].T @ S[128e,32]
    where S is a norm-weighted one-hot built on DVE from uploaded shift/weight
    arrays (2 fused DVE ops per superblock).
  - layer matmul, bias, relu, log-softmax fused on PE/DVE/ACT; AllGather
    (collective) redistributes shards between layers.
  - reg_loss (pure function of edge_index) via sorted key lookup on host.
"""

import sys

sys.path.insert(0, "/opt/trn_rl_repo")

import numpy as np

N = 40000
E = 640000
D = 128
DOUT = 40
NCORES = 8
SHARD = N // NCORES  # 5000
WIN = 32  # nodes per segsum window
CHUNK = 128  # edge slots per chunk (matmul contraction dim)
LO_ROWS = 32768  # rows addressable from base row 0 (int16 idx)
HI_BASE = N - LO_ROWS  # 7232; hi gather base row
SB_CHUNK_BUDGET = 56  # max chunks per superblock (gather piece)
NWIN = (SHARD + WIN - 1) // WIN  # 157 windows/core (last has 8 nodes)
NBLK = (SHARD + 127) // 128  # 40 matmul2 node blocks (last has 8)


def _cdiv(a, b):
    return (a + b - 1) // b


def _prep(edge_index):
    """Host-side index preprocessing: sharding, windowing, gather/selection arrays."""
    src = np.asarray(edge_index[0]).astype(np.int64)
    dst = np.asarray(edge_index[1]).astype(np.int64)

    deg = (np.bincount(dst, minlength=N) + 1).astype(np.float32)  # self-loop +1
    dis = 1.0 / np.sqrt(deg.astype(np.float64))

    # degree-rank round-robin node -> (core, pos)
    order = np.argsort(-deg, kind="stable")  # node ids, degree desc
    core_of = np.empty(N, np.int64)
    pos_of = np.empty(N, np.int64)
    r = np.arange(N)
    core_of[order] = r % NCORES
    pos_of[order] = r // NCORES
    g_of = core_of * SHARD + pos_of  # row in the replicated-H layout
    perm = np.empty(N, np.int64)  # perm[g] = original node id
    perm[g_of] = np.arange(N)

    # edge instances incl. self-loops
    es = np.concatenate([src, np.arange(N)])
    ed = np.concatenate([dst, np.arange(N)])
    ew = (dis[es] * dis[ed]).astype(np.float32)
    ecore = core_of[ed]
    ewin = pos_of[ed] // WIN
    eshift = (pos_of[ed] - ewin * WIN).astype(np.float32)
    egsrc = g_of[es]

    # class: 0 = strict-lo (g < HI_BASE), 1 = mid, 2 = strict-hi (g >= LO_ROWS)
    ecls = np.ones(len(es), np.int64)
    ecls[egsrc < HI_BASE] = 0
    ecls[egsrc >= LO_ROWS] = 2

    per_core = []
    cnt = np.zeros((NCORES, NWIN, 3), np.int64)
    for c in range(NCORES):
        m = ecore == c
        o = np.lexsort((ecls[m], ewin[m]))
        cw = {
            "win": ewin[m][o],
            "cls": ecls[m][o],
            "gsrc": egsrc[m][o],
            "shift": eshift[m][o],
            "w": ew[m][o],
        }
        per_core.append(cw)
        for k in range(3):
            np.add.at(cnt[c], (cw["win"][cw["cls"] == k], k), 1)

    # uniform chunk counts per window
    K_lo = np.max(_cdiv(cnt[:, :, 0], CHUNK), axis=0)  # [NWIN]
    n_tot = cnt.sum(axis=2)  # [NCORES, NWIN]
    lo_fill = np.minimum(cnt[:, :, 0] + cnt[:, :, 1], K_lo[None, :] * CHUNK)
    hi_cnt = n_tot - lo_fill
    K_hi = np.max(_cdiv(hi_cnt, CHUNK), axis=0)  # [NWIN]

    # superblocks: consecutive windows, chunk budget
    sbs = []
    w = 0
    while w < NWIN:
        w0 = w
        ch = 0
        wins = []
        while w < NWIN and (ch == 0 or ch + K_lo[w] + K_hi[w] <= SB_CHUNK_BUDGET):
            ch += int(K_lo[w] + K_hi[w])
            wins.append(w)
            w += 1
        sbs.append({"w0": w0, "wins": wins})
    # global chunk layout: per sb: [lo chunks of wins | hi chunks of wins]
    gch = 0
    for sb in sbs:
        sb["lo_c0"] = gch
        lo_off = []
        for w_ in sb["wins"]:
            lo_off.append(gch)
            gch += int(K_lo[w_])
        sb["hi_c0"] = gch
        hi_off = []
        for w_ in sb["wins"]:
            hi_off.append(gch)
            gch += int(K_hi[w_])
        sb["lo_off"] = lo_off
        sb["hi_off"] = hi_off
        sb["lo_nch"] = sb["hi_c0"] - sb["lo_c0"]
        sb["hi_nch"] = gch - sb["hi_c0"]
    NCH = gch
    NSLOT = NCH * CHUNK

    # fill per-core slot arrays
    idx_flat = np.zeros((NCORES, NSLOT), np.int32)
    shift_flat = np.full((NCORES, NSLOT), -1.0, np.float32)
    wgt_flat = np.zeros((NCORES, NSLOT), np.float32)
    for c in range(NCORES):
        cw = per_core[c]
        wstart = np.searchsorted(cw["win"], np.arange(NWIN))
        wend = np.searchsorted(cw["win"], np.arange(NWIN), side="right")
        for sb in sbs:
            for j, w_ in enumerate(sb["wins"]):
                a, b = int(wstart[w_]), int(wend[w_])
                nlo = int(lo_fill[c, w_])
                nhi = b - a - nlo
                s0 = sb["lo_off"][j] * CHUNK
                idx_flat[c, s0 : s0 + nlo] = cw["gsrc"][a : a + nlo]
                shift_flat[c, s0 : s0 + nlo] = cw["shift"][a : a + nlo]
                wgt_flat[c, s0 : s0 + nlo] = cw["w"][a : a + nlo]
                s1 = sb["hi_off"][j] * CHUNK
                idx_flat[c, s1 : s1 + nhi] = cw["gsrc"][a + nlo : b] - HI_BASE
                shift_flat[c, s1 : s1 + nhi] = cw["shift"][a + nlo : b]
                wgt_flat[c, s1 : s1 + nhi] = cw["w"][a + nlo : b]
                assert np.all(cw["gsrc"][a : a + nlo] < LO_ROWS)
                assert np.all(cw["gsrc"][a + nlo : b] >= HI_BASE)

    assert idx_flat.min() >= 0 and idx_flat.max() < LO_ROWS

    # pack: idx 16-wrapped replicated down partitions; shift/wgt 128-wrapped
    COLS = NSLOT // 16
    idx16 = np.zeros((NCORES, 128, COLS), np.int16)
    for c in range(NCORES):
        t = idx_flat[c].reshape(COLS, 16).T.astype(np.int16)  # [16, COLS]
        idx16[c] = np.tile(t, (8, 1))
    shift_t = shift_flat.reshape(NCORES, NCH, CHUNK).transpose(0, 2, 1).copy()
    wgt_t = wgt_flat.reshape(NCORES, NCH, CHUNK).transpose(0, 2, 1).copy()

    sb_max = max(sb["lo_nch"] + sb["hi_nch"] for sb in sbs)
    iota_rep = np.tile(np.arange(WIN, dtype=np.float32), (128, sb_max))

    return {
        "perm": perm,
        "g_of": g_of,
        "sbs": sbs,
        "K_lo": K_lo,
        "K_hi": K_hi,
        "NCH": NCH,
        "NSLOT": NSLOT,
        "COLS": COLS,
        "sb_max": sb_max,
        "idx16": idx16,
        "shift_t": shift_t,
        "wgt_t": wgt_t,
        "iota_rep": iota_rep,
    }


def _emulate(prep, xp, W1, b1, W2, b2, W3, b3):
    """Numpy emulation mirroring the device program exactly (for validation)."""
    sbs = prep["sbs"]
    iota = np.arange(WIN, dtype=np.float32)
    h = xp.astype(np.float32)  # replicated layer input [N, D]
    for l, (W, b) in enumerate([(W1, b1), (W2, b2), (W3, b3)]):
        shards = []
        for c in range(NCORES):
            idx = prep["idx16"][c][:16].T.reshape(-1).astype(np.int64)  # interp unwrap
            G_T = np.zeros((D, SHARD), np.float32)
            for sb in sbs:
                for j, w_ in enumerate(sb["wins"]):
                    psum = np.zeros((D, WIN), np.float32)
                    chunks = [sb["lo_off"][j] + k for k in range(prep["K_lo"][w_])] + [
                        sb["hi_off"][j] + k for k in range(prep["K_hi"][w_])
                    ]
                    for cg in chunks:
                        base = 0 if cg < sb["hi_c0"] else HI_BASE
                        rows = idx[cg * CHUNK : (cg + 1) * CHUNK].astype(np.int64) + base
                        msg = h[rows]  # [128, D]
                        sh = prep["shift_t"][c][:, cg]  # [128]
                        wv = prep["wgt_t"][c][:, cg]
                        S = (iota[None, :] == sh[:, None]) * wv[:, None]
                        psum += msg.T @ S.astype(np.float32)
                    n0 = w_ * WIN
                    nn = min(WIN, SHARD - n0)
                    G_T[:, n0 : n0 + nn] += psum[:, :nn]
            Z = G_T.T @ W + b[None, :]  # [SHARD, dout]
            if l < 2:
                shards.append(np.maximum(Z, 0.0))
            else:
                m = Z.max(axis=1, keepdims=True)
                e = np.exp(Z - m)
                shards.append(Z - m - np.log(e.sum(axis=1, keepdims=True)))
        h = np.concatenate(shards, axis=0)
    return h  # [N, dout] in permuted (g) order


def _reg_loss(edge_index):
    src = np.asarray(edge_index[0]).astype(np.int64)
    dst = np.asarray(edge_index[1]).astype(np.int64)
    keys = np.sort(src * N + dst)
    rev = dst * N + src
    lo = np.searchsorted(keys, rev, side="left")
    hi = np.searchsorted(keys, rev, side="right")
    return np.float32((hi - lo).sum())


# ---------------------------------------------------------------- device code


def _build(prep, n_layers=3, do_gather=True, do_collective=True):
    from contextlib import ExitStack

    import concourse.bacc as bacc
    import concourse.mybir as mybir
    import concourse.tile as tile

    F32 = mybir.dt.float32
    I16 = mybir.dt.int16
    OP = mybir.AluOpType
    ACTF = mybir.ActivationFunctionType

    sbs = prep["sbs"]
    K_lo, K_hi = prep["K_lo"], prep["K_hi"]
    NCH, COLS, sb_max = prep["NCH"], prep["COLS"], prep["sb_max"]

    nc = bacc.Bacc("TRN2", target_bir_lowering=False, debug=False, num_devices=NCORES)

    xp_d = nc.dram_tensor("xp", [N, D], F32, kind="ExternalInput")
    idx_d = nc.dram_tensor("idx", [128, COLS], I16, kind="ExternalInput")
    shift_d = nc.dram_tensor("shift", [128, NCH], F32, kind="ExternalInput")
    wgt_d = nc.dram_tensor("wgt", [128, NCH], F32, kind="ExternalInput")
    iota_d = nc.dram_tensor("iota", [128, sb_max * WIN], F32, kind="ExternalInput")
    w_d = [
        nc.dram_tensor("w1", [D, D], F32, kind="ExternalInput"),
        nc.dram_tensor("w2", [D, D], F32, kind="ExternalInput"),
        nc.dram_tensor("w3", [D, DOUT], F32, kind="ExternalInput"),
    ]
    b_d = [
        nc.dram_tensor("b1", [1, D], F32, kind="ExternalInput"),
        nc.dram_tensor("b2", [1, D], F32, kind="ExternalInput"),
        nc.dram_tensor("b3", [1, DOUT], F32, kind="ExternalInput"),
    ]
    out_d = nc.dram_tensor("out", [SHARD, DOUT], F32, kind="ExternalOutput")
    hrepl = [
        nc.dram_tensor(f"hrepl{i}", [N, D], F32, addr_space="Shared")
        for i in range(2)
    ]
    shard_d = [nc.dram_tensor(f"shard{i}", [SHARD, D], F32) for i in range(2)]

    douts = [D, D, DOUT]

    with tile.TileContext(nc) as tc, ExitStack() as ctx:
        const = ctx.enter_context(tc.tile_pool(name="const", bufs=1))
        msg_pool = ctx.enter_context(tc.tile_pool(name="msg", bufs=2))
        s_pool = ctx.enter_context(tc.tile_pool(name="sgen", bufs=2))
        gt_pool = ctx.enter_context(tc.tile_pool(name="gt", bufs=1))
        hout_pool = ctx.enter_context(tc.tile_pool(name="hout", bufs=3))
        p1_pool = ctx.enter_context(tc.tile_pool(name="p1", bufs=4, space="PSUM"))
        p2_pool = ctx.enter_context(tc.tile_pool(name="p2", bufs=2, space="PSUM"))

        # ---- constants
        idx_sb = const.tile([128, COLS], I16, tag="idx")
        nc.sync.dma_start(idx_sb[:], idx_d[:])
        shift_sb = const.tile([128, NCH], F32, tag="shift")
        nc.sync.dma_start(shift_sb[:], shift_d[:])
        wgt_sb = const.tile([128, NCH], F32, tag="wgt")
        nc.sync.dma_start(wgt_sb[:], wgt_d[:])
        iota_sb = const.tile([128, sb_max * WIN], F32, tag="iota")
        nc.sync.dma_start(iota_sb[:], iota_d[:])
        w_sb = []
        b_row = []
        for l in range(3):
            t = const.tile([D, douts[l]], F32, tag=f"w{l}")
            nc.sync.dma_start(t[:], w_d[l][:])
            w_sb.append(t)
            t = const.tile([1, douts[l]], F32, tag=f"b{l}")
            nc.sync.dma_start(t[:], b_d[l][:])
            b_row.append(t)
        # bias broadcast tiles via PE outer product with a ones column
        ones_sb = const.tile([1, 128], F32, tag="ones")
        nc.vector.memset(ones_sb[:], 1.0)
        b_bc = []
        for l in range(3):
            pb = p2_pool.tile([128, douts[l]], F32, space="PSUM")
            nc.tensor.matmul(
                out=pb[:], lhsT=ones_sb[:], rhs=b_row[l][:], start=True, stop=True
            )
            t = const.tile([128, douts[l]], F32, tag=f"bbc{l}")
            nc.scalar.copy(t[:], pb[:])
            b_bc.append(t)

        for l in range(n_layers):
            src_dram = xp_d if l == 0 else hrepl[l - 1]
            G_T = gt_pool.tile([128, SHARD], F32, tag="gt")
            if not do_gather:
                nc.vector.memset(G_T[:], 0.0)
            for sb in sbs if do_gather else []:
                lo_nch, hi_nch = sb["lo_nch"], sb["hi_nch"]
                nch = lo_nch + hi_nch
                c0 = sb["lo_c0"]
                msg = msg_pool.tile([128, nch * D], F32, tag="msg")
                msg3 = msg[:].rearrange("p (c d) -> p c d", d=D)
                if lo_nch:
                    nc.gpsimd.dma_gather(
                        out_ap=msg3[:, 0:lo_nch, :],
                        in_ap=src_dram[0:LO_ROWS, :],
                        idxs_ap=idx_sb[:, c0 * 8 : (c0 + lo_nch) * 8],
                        num_idxs=lo_nch * CHUNK,
                        num_idxs_reg=lo_nch * CHUNK,
                        elem_size=D,
                        single_packet=False,
                    )
                if hi_nch:
                    nc.gpsimd.dma_gather(
                        out_ap=msg3[:, lo_nch:nch, :],
                        in_ap=src_dram[HI_BASE:N, :],
                        idxs_ap=idx_sb[:, (c0 + lo_nch) * 8 : (c0 + nch) * 8],
                        num_idxs=hi_nch * CHUNK,
                        num_idxs_reg=hi_nch * CHUNK,
                        elem_size=D,
                        single_packet=False,
                    )
                # S generation (2 fused DVE ops over the whole superblock)
                t_t = s_pool.tile([128, nch * WIN], F32, tag="tgen")
                s_t = s_pool.tile([128, nch * WIN], F32, tag="sgen")
                t3 = t_t[:].rearrange("p (c w) -> p c w", w=WIN)
                s3 = s_t[:].rearrange("p (c w) -> p c w", w=WIN)
                io3 = iota_sb[:, 0 : nch * WIN].rearrange("p (c w) -> p c w", w=WIN)
                sh_bc = shift_sb[:, c0 : c0 + nch, None].to_broadcast((128, nch, WIN))
                wg_bc = wgt_sb[:, c0 : c0 + nch, None].to_broadcast((128, nch, WIN))
                nc.vector.tensor_tensor(t3, io3, sh_bc, OP.subtract)
                nc.vector.scalar_tensor_tensor(
                    s3, t3, 0.0, wg_bc, OP.is_equal, OP.mult
                )
                # per-window accumulation
                for j, w_ in enumerate(sb["wins"]):
                    chunks = [sb["lo_off"][j] - c0 + k for k in range(K_lo[w_])] + [
                        sb["hi_off"][j] - c0 + k for k in range(K_hi[w_])
                    ]
                    psum1 = p1_pool.tile([128, WIN], F32, space="PSUM")
                    for i, cl in enumerate(chunks):
                        nc.tensor.matmul(
                            out=psum1[:],
                            lhsT=msg[:, cl * D : (cl + 1) * D],
                            rhs=s_t[:, cl * WIN : (cl + 1) * WIN],
                            start=(i == 0),
                            stop=(i == len(chunks) - 1),
                        )
                    n0 = w_ * WIN
                    nn = min(WIN, SHARD - n0)
                    nc.scalar.copy(G_T[:, n0 : n0 + nn], psum1[:, :nn])
            # ---- layer matmul + bias (+relu / +log_softmax) ----
            if l < 2:
                for nb in range(NBLK):
                    m = min(128, SHARD - nb * 128)
                    psum2 = p2_pool.tile([128, D], F32, space="PSUM")
                    nc.tensor.matmul(
                        out=psum2[:m, :],
                        lhsT=G_T[:, nb * 128 : nb * 128 + m],
                        rhs=w_sb[l][:],
                        start=True,
                        stop=True,
                    )
                    tsum = hout_pool.tile([128, D], F32, tag="tsum")
                    nc.vector.tensor_tensor(
                        tsum[:m, :], psum2[:m, :], b_bc[l][:m, :], OP.add
                    )
                    hout = hout_pool.tile([128, D], F32, tag="hout")
                    nc.scalar.activation(hout[:m, :], tsum[:m, :], ACTF.Relu)
                    nc.sync.dma_start(
                        shard_d[l][nb * 128 : nb * 128 + m, :], hout[:m, :]
                    )
                if do_collective:
                    tc.strict_bb_all_engine_barrier()
                    nc.gpsimd.collective_compute(
                        "AllGather",
                        OP.bypass,
                        replica_groups=[list(range(NCORES))],
                        ins=[shard_d[l][:]],
                        outs=[hrepl[l][:]],
                    )
                    tc.strict_bb_all_engine_barrier()
                else:
                    nc.sync.dma_start(hrepl[l][l * SHARD : (l + 1) * SHARD, :], shard_d[l][:])
            else:
                zt = const.tile([128, NBLK * DOUT], F32, tag="zt")
                nc.vector.memset(zt[:], 0.0)
                for nb in range(NBLK):
                    m = min(128, SHARD - nb * 128)
                    psum2 = p2_pool.tile([128, DOUT], F32, space="PSUM")
                    nc.tensor.matmul(
                        out=psum2[:m, :],
                        lhsT=G_T[:, nb * 128 : nb * 128 + m],
                        rhs=w_sb[l][:],
                        start=True,
                        stop=True,
                    )
                    nc.vector.tensor_tensor(
                        zt[:m, nb * DOUT : (nb + 1) * DOUT],
                        psum2[:m, :],
                        b_bc[l][:m, :],
                        OP.add,
                    )
                z3 = zt[:].rearrange("p (n c) -> p n c", c=DOUT)
                mx = const.tile([128, NBLK], F32, tag="mx")
                nc.vector.tensor_reduce(mx[:], z3, mybir.AxisListType.X, OP.max, negate=True)
                sub = const.tile([128, NBLK * DOUT], F32, tag="sub")
                sub3 = sub[:].rearrange("p (n c) -> p n c", c=DOUT)
                nc.vector.scalar_tensor_tensor(
                    sub3,
                    z3,
                    0.0,
                    mx[:, :, None].to_broadcast((128, NBLK, DOUT)),
                    OP.bypass,
                    OP.add,
                )
                ex = const.tile([128, NBLK * DOUT], F32, tag="ex")
                nc.scalar.activation(ex[:], sub[:], ACTF.Exp)
                sm = const.tile([128, NBLK], F32, tag="sm")
                nc.vector.tensor_reduce(
                    sm[:],
                    ex[:].rearrange("p (n c) -> p n c", c=DOUT),
                    mybir.AxisListType.X,
                    OP.add,
                )
                ls = const.tile([128, NBLK], F32, tag="ls")
                nc.scalar.activation(ls[:], sm[:], ACTF.Ln)
                fin = const.tile([128, NBLK * DOUT], F32, tag="fin")
                fin3 = fin[:].rearrange("p (n c) -> p n c", c=DOUT)
                nc.vector.scalar_tensor_tensor(
                    fin3,
                    sub3,
                    0.0,
                    ls[:, :, None].to_broadcast((128, NBLK, DOUT)),
                    OP.bypass,
                    OP.subtract,
                )
                for nb in range(NBLK):
                    m = min(128, SHARD - nb * 128)
                    nc.sync.dma_start(
                        out_d[nb * 128 : nb * 128 + m, :],
                        fin[:m, nb * DOUT : (nb + 1) * DOUT],
                    )
        if n_layers < 3:
            nc.sync.dma_start(out_d[:], shard_d[n_layers - 1][:, 0:DOUT])
    nc.finalize()
    return nc


_CACHE = {}


def _get_program(edge_index):
    key = hash(np.asarray(edge_index).tobytes())
    if key not in _CACHE:
        prep = _prep(edge_index)
        nc = _build(prep)
        _CACHE[key] = (prep, nc)
    return _CACHE[key]


def _ensure_axon_hooks_stub():
    """bass_utils unconditionally imports antenv.axon_hooks under BASS_TRACE;
    provide a no-op registry if the container's antenv stub lacks it."""
    try:
        import antenv.axon_hooks  # noqa: F401
    except ImportError:
        import types

        import antenv

        m = types.ModuleType("antenv.axon_hooks")
        m._H = None
        m.set_axon_ntff_profile_hook = lambda h: setattr(m, "_H", h)
        m.get_axon_ntff_profile_hook = lambda: m._H
        sys.modules["antenv.axon_hooks"] = m
        antenv.axon_hooks = m


def kernel(x, edge_index, W1, b1, W2, b2, W3, b3):
    _ensure_axon_hooks_stub()
    from concourse.bass_utils import run_bass_kernel_spmd

    x = np.ascontiguousarray(np.asarray(x, dtype=np.float32))
    prep, nc = _get_program(edge_index)

    xp = np.ascontiguousarray(x[prep["perm"]])
    shared = {
        "xp": xp,
        "iota": prep["iota_rep"],
        "w1": np.ascontiguousarray(np.asarray(W1, np.float32)),
        "w2": np.ascontiguousarray(np.asarray(W2, np.float32)),
        "w3": np.ascontiguousarray(np.asarray(W3, np.float32)),
        "b1": np.ascontiguousarray(np.asarray(b1, np.float32).reshape(1, D)),
        "b2": np.ascontiguousarray(np.asarray(b2, np.float32).reshape(1, D)),
        "b3": np.ascontiguousarray(np.asarray(b3, np.float32).reshape(1, DOUT)),
    }
    in_maps = []
    for c in range(NCORES):
        m = dict(shared)
        m["idx"] = np.ascontiguousarray(prep["idx16"][c])
        m["shift"] = np.ascontiguousarray(prep["shift_t"][c])
        m["wgt"] = np.ascontiguousarray(prep["wgt_t"][c])
        in_maps.append(m)

    res = run_bass_kernel_spmd(nc, in_maps, core_ids=list(range(NCORES)))
    global LAST_RESULTS
    LAST_RESULTS = res
    out_p = np.concatenate([res.results[c]["out"] for c in range(NCORES)], axis=0)
    out = np.empty((N, DOUT), np.float32)
    out[prep["perm"]] = out_p
    return out, _reg_loss(edge_index)


def _np_reference(x, edge_index, W1, b1, W2, b2, W3, b3):
    src = np.asarray(edge_index[0]).astype(np.int64)
    dst = np.asarray(edge_index[1]).astype(np.int64)
    loops = np.arange(N)
    src2 = np.concatenate([src, loops])
    dst2 = np.concatenate([dst, loops])
    deg = np.bincount(dst2, minlength=N).astype(np.float32)
    dis = np.where(deg > 0, 1.0 / np.sqrt(deg), 0.0).astype(np.float32)
    norm = dis[src2] * dis[dst2]

    def layer(h, W, b):
        hw = h @ W
        msg = norm[:, None] * hw[src2]
        out = np.zeros((N, W.shape[1]), np.float32)
        np.add.at(out, dst2, msg)
        return out + b[None, :]

    h = np.maximum(layer(np.asarray(x, np.float32), W1, b1), 0)
    h = np.maximum(layer(h, W2, b2), 0)
    z = layer(h, W3, b3)
    m = z.max(axis=1, keepdims=True)
    ls = z - m - np.log(np.exp(z - m).sum(axis=1, keepdims=True))
    return ls


if __name__ == "__main__":
    rng = np.random.default_rng(0)
    inputs = {
        "x": rng.standard_normal((N, D), dtype=np.float32),
        "edge_index": rng.integers(0, N, size=(2, E), dtype=np.int64),
        "W1": (rng.standard_normal((D, D), dtype=np.float32) / np.sqrt(D)),
        "b1": np.zeros(D, np.float32),
        "W2": (rng.standard_normal((D, D), dtype=np.float32) / np.sqrt(D)),
        "b2": np.zeros(D, np.float32),
        "W3": (rng.standard_normal((D, DOUT), dtype=np.float32) / np.sqrt(D)),
        "b3": np.zeros(DOUT, np.float32),
    }
    exp_logits = _np_reference(**inputs)

    prep = _prep(inputs["edge_index"])
    print(
        f"NCH={prep['NCH']} NSLOT={prep['NSLOT']} sb_max={prep['sb_max']} "
        f"n_sbs={len(prep['sbs'])} pad_frac={1 - (E + N) / prep['NSLOT']:.3f}"
    )
    xp = inputs["x"][prep["perm"]]
    out_p = _emulate(
        prep, xp, inputs["W1"], inputs["b1"], inputs["W2"], inputs["b2"],
        inputs["W3"], inputs["b3"],
    )
    out = np.empty((N, DOUT), np.float32)
    out[prep["perm"]] = out_p
    err = np.abs(out - exp_logits).max() / (np.abs(exp_logits).max() + 1e-9)
    rel = np.linalg.norm(out - exp_logits) / np.linalg.norm(exp_logits)
    print(f"emulator: max_abs_rel={err:.3e} fro_rel={rel:.3e}")
